# revision 6
# baseline (speedup 1.0000x reference)
"""TRN2 Bass kernel for nn_Attention_15590731285136.

Computation (per batch b):
    g      = diag(W) * K[b]                       # [d]
    score  = relu(V[b] @ (g[:,None]*w1) + b1) @ w2 + b2   # [h]
    score  = where(mask[b], MASK_FILL, score)
    alpha  = softmax(score)                        # over h
    out[b] = alpha @ V[b]                          # [d]

Sharding: data-parallel over batch, 8 batches per core on 8 NeuronCores.

Key transformations:
  * The elementwise gate folds into the weight matrix: V*g @ w1 = V @ (g[:,None]*w1).
  * w2 folds into w1's columns by |w2| with a sign-grouping permutation, so the
    w2-dot becomes two plain row-sums of the relu output; those are computed
    for free by the fused relu+accumulate paths on ScalarE (ACT) and VectorE.
  * V is pre-cast to bf16 on the host; the d-major (transposed) copy needed for
    the fc1 contraction is produced by the DMA xbar transpose during the load,
    so the PE runs only the essential matmuls.
  * softmax skips max-subtraction (scores are O(0.1); masked entries get an
    additive -2^32 bias so exp underflows to exactly 0); normalization happens
    once at the end on the [1, 512] pooled accumulator.
"""

import numpy as np

B, H, D, HID = 64, 2048, 512, 512
NCORES = 8
BPC = B // NCORES          # batches per core
HT = H // 128              # 16 h-tiles per batch
DC = D // 128              # 4 contraction chunks
MASK_FILL = -2.0**32 + 1.0


def _build(hp, b2val, has_bias):
    import concourse.mybir as mybir
    from concourse import bacc
    from concourse.tile import TileContext

    F32 = mybir.dt.float32
    F16 = mybir.dt.float16
    ACTF = mybir.ActivationFunctionType
    ALU = mybir.AluOpType

    nc = bacc.Bacc(trn_type="TRN2", num_devices=NCORES)

    VB = nc.dram_tensor("VB", (BPC, H, D), F16, kind="ExternalInput")
    GT = nc.dram_tensor("GT", (BPC, 128, DC), F32, kind="ExternalInput")
    MB = nc.dram_tensor("MB", (BPC, 128, HT), F32, kind="ExternalInput")
    WA = nc.dram_tensor("WA", (D, HID), F32, kind="ExternalInput")
    if has_bias:
        BI = nc.dram_tensor("BI", (1, HID), F32, kind="ExternalInput")
    OUT = nc.dram_tensor("OUT", (BPC, D), F32, kind="ExternalOutput")

    with TileContext(nc) as tc:
        with (
            tc.tile_pool(name="const", bufs=1) as cpool,
            tc.tile_pool(name="v", bufs=2) as vpool,
            tc.tile_pool(name="vt", bufs=2 * DC) as vtpool,
            tc.tile_pool(name="w12", bufs=2) as wpool,
            tc.tile_pool(name="small", bufs=2) as spool,
            tc.tile_pool(name="scr", bufs=2) as scrpool,
            tc.tile_pool(name="fin", bufs=2) as finpool,
            tc.tile_pool(name="fc1_ps", bufs=2, space="PSUM") as fc1ps,
            tc.tile_pool(name="tot_ps", bufs=2, space="PSUM") as totps,
            tc.tile_pool(name="acc_ps", bufs=2, space="PSUM") as accps,
        ):
            # ---- one-time constants ----
            ones_col = cpool.tile([128, 1], F16, tag="ones")
            nc.vector.memset(ones_col, 1.0)

            # WA as [128, DC*HID]: chunk c at cols [c*HID, (c+1)*HID)
            wabs = cpool.tile([128, DC * HID], F32, tag="wabs")
            nc.sync.dma_start(
                out=wabs.rearrange("p (c n) -> p c n", c=DC),
                in_=WA.ap().rearrange("(c p) n -> p c n", p=128),
            )
            if has_bias:
                ones_row = cpool.tile([1, 128], F16, tag="orr")
                nc.vector.memset(ones_row, 1.0)
                bias_sb = cpool.tile([1, HID], F16, tag="bias")
                bias_f = cpool.tile([1, HID], F32, tag="biasf")
                nc.sync.dma_start(out=bias_f, in_=BI.ap())
                nc.vector.tensor_copy(bias_sb, bias_f)

            for bi in range(BPC):
                # ---- per-batch small loads ----
                gcol = spool.tile([128, DC], F32, tag="gcol")
                nc.sync.dma_start(out=gcol, in_=GT.ap()[bi])
                mb = spool.tile([128, HT], F32, tag="mb")
                nc.sync.dma_start(out=mb, in_=MB.ap()[bi])

                # ---- gate the packed weights: W12[d, :] = g[d] * Wabs[d, :] ----
                w12 = wpool.tile([128, DC * HID], F16, tag="w12")
                for c in range(DC):
                    nc.vector.tensor_scalar_mul(
                        w12[:, c * HID:(c + 1) * HID],
                        wabs[:, c * HID:(c + 1) * HID],
                        gcol[:, c:c + 1],
                    )

                # ---- V[bi]^T via DMA xbar transpose: [128 d, 2048 tok] per chunk ----
                vts = []
                for c in range(DC):
                    vt = vtpool.tile([128, H], F16, tag="vt")
                    nc.sync.dma_start(
                        out=vt,
                        in_=VB.ap()[bi, :, c * 128:(c + 1) * 128],
                        transpose=True,
                    )
                    vts.append(vt)

                # ---- V[bi] natural [128 tok, j, d] (pass-2 rhs), 4 DMAs ----
                v_all = vpool.tile([128, HT * D], F16, tag="v")
                v3 = v_all.rearrange("p (j d) -> p j d", j=HT)
                for q in range(4):
                    nc.sync.dma_start(
                        out=v3[:, 4 * q:4 * q + 4, :],
                        in_=VB.ap()[bi, 512 * q:512 * (q + 1), :]
                            .rearrange("(j p) d -> p j d", p=128),
                    )

                sp = spool.tile([128, HT], F32, tag="sp")
                sn = spool.tile([128, HT], F32, tag="sn")
                if hp == 0:
                    nc.vector.memset(sp, 0.0)
                if hp == HID:
                    nc.vector.memset(sn, 0.0)

                # ---- fc1 + fused relu/rowsum per tok-tile ----
                for j in range(HT):
                    fc1 = fc1ps.tile([128, HID], F32, tag="fc1")
                    for c in range(DC):
                        nc.tensor.matmul(
                            out=fc1,
                            lhsT=vts[c][:, j * 128:(j + 1) * 128],
                            rhs=w12[:, c * HID:(c + 1) * HID],
                            start=(c == 0),
                            stop=(c == DC - 1) and not has_bias,
                        )
                    if has_bias:
                        nc.tensor.matmul(
                            out=fc1, lhsT=ones_row, rhs=bias_sb,
                            start=False, stop=True,
                        )
                    # positive-w2 half on ACT (fused relu+rowsum)...
                    if hp > 0:
                        scra = scrpool.tile([128, HID], F16, tag="scra")
                        nc.scalar.activation(
                            out=scra[:, :hp], in_=fc1[:, :hp], func=ACTF.Relu,
                            accum_out=sp[:, j:j + 1],
                        )
                    # ...negative-w2 half on DVE (max(x,0) + add-reduce)
                    if hp < HID:
                        scrd = scrpool.tile([128, HID], F16, tag="scrd")
                        nc.vector.tensor_scalar(
                            out=scrd[:, hp:], in0=fc1[:, hp:],
                            scalar1=0.0, scalar2=None,
                            op0=ALU.max, op1=ALU.add,
                            accum_out=sn[:, j:j + 1],
                        )

                # ---- scores -> masked -> exp ----
                sc = spool.tile([128, HT], F32, tag="sc")
                nc.vector.tensor_sub(sc, sp, sn)
                scm = spool.tile([128, HT], F32, tag="scm")
                nc.vector.tensor_add(scm, sc, mb)
                alpha = spool.tile([128, HT], F16, tag="alpha")
                nc.scalar.activation(
                    out=alpha, in_=scm, func=ACTF.Exp, bias=float(b2val),
                )

                # ---- denominator: sum over all tokens ----
                tot = totps.tile([1, HT], F32, tag="tot")
                nc.tensor.matmul(out=tot, lhsT=ones_col, rhs=alpha,
                                 start=True, stop=True)
                tot_sb = finpool.tile([1, 1], F32, tag="tot_sb")
                nc.vector.tensor_reduce(
                    tot_sb, tot, axis=mybir.AxisListType.X, op=ALU.add,
                )
                inv = finpool.tile([1, 1], F32, tag="inv")
                nc.vector.reciprocal(inv, tot_sb)

                # ---- pass 2: acc[1, d] = sum_j alpha[:, j]^T @ V_j ----
                acc = accps.tile([1, D], F32, tag="acc")
                for j in range(HT):
                    nc.tensor.matmul(
                        out=acc,
                        lhsT=alpha[:, j:j + 1],
                        rhs=v3[:, j, :],
                        start=(j == 0),
                        stop=(j == HT - 1),
                    )
                ob = finpool.tile([1, D], F32, tag="ob")
                nc.vector.tensor_scalar_mul(ob, acc, inv)
                nc.sync.dma_start(out=OUT.ap()[bi:bi + 1, :], in_=ob)

    nc.finalize()
    return nc


def _prep(K, V, mask, W, w1, b1, w2, b2):
    """Host-side input marshalling (no heavy compute)."""
    import ml_dtypes

    K = np.asarray(K, dtype=np.float32)
    V = np.asarray(V, dtype=np.float32)
    mask = np.asarray(mask)
    W = np.asarray(W, dtype=np.float32)
    w1 = np.asarray(w1, dtype=np.float32)
    b1 = np.asarray(b1, dtype=np.float32)
    w2 = np.asarray(w2, dtype=np.float32).reshape(-1)
    b2 = np.asarray(b2, dtype=np.float32).reshape(-1)

    Vb = np.ascontiguousarray(V.astype(np.float16))

    g = np.diagonal(W).astype(np.float32) * K          # [B, D]
    pos = w2 >= 0.0
    perm = np.argsort(~pos, kind="stable")             # positives first
    hp = int(pos.sum())
    wabs = (w1[:, perm] * np.abs(w2[perm])[None, :]).astype(np.float32)
    bias12 = (b1[perm] * np.abs(w2[perm])).astype(np.float32)
    has_bias = bool(np.any(bias12 != 0.0))

    # g arranged [B, 128, DC] so chunk c sits in column c (partition-major)
    gt = np.ascontiguousarray(g.reshape(B, DC, 128).transpose(0, 2, 1))
    # additive mask bias [B, 128, HT]: token j*128+p -> [p, j]
    mbias = np.where(mask, np.float32(MASK_FILL), np.float32(0.0)).astype(np.float32)
    mbias = np.ascontiguousarray(mbias.reshape(B, HT, 128).transpose(0, 2, 1))
    return Vb, gt, mbias, wabs, bias12, has_bias, hp, float(b2[0]) if b2.size else 0.0


def kernel(K, V, mask, W, w1, b1, w2, b2):
    from concourse import bass_utils

    Vb, gt, mbias, wabs, bias12, has_bias, hp, b2val = _prep(
        K, V, mask, W, w1, b1, w2, b2
    )
    nc = _build(hp, b2val, has_bias)

    in_maps = []
    for c in range(NCORES):
        sl = slice(c * BPC, (c + 1) * BPC)
        m = {
            "VB": Vb[sl],
            "GT": gt[sl],
            "MB": mbias[sl],
            "WA": wabs,
        }
        if has_bias:
            m["BI"] = bias12.reshape(1, HID)
        in_maps.append(m)

    res = bass_utils.run_bass_kernel_spmd(nc, in_maps, core_ids=list(range(NCORES)))
    out = np.concatenate([res.results[c]["OUT"] for c in range(NCORES)], axis=0)
    return out.astype(np.float32)


# revision 9
# speedup vs baseline: 1.1143x; 1.1143x over previous
"""TRN2 Bass kernel for nn_Attention_15590731285136.

Computation (per batch b):
    g      = diag(W) * K[b]                       # [d]
    score  = relu(V[b] @ (g[:,None]*w1) + b1) @ w2 + b2   # [h]
    score  = where(mask[b], MASK_FILL, score)
    alpha  = softmax(score)                        # over h
    out[b] = alpha @ V[b]                          # [d]

Sharding: data-parallel over batch, 8 batches per core on 8 NeuronCores.

Key transformations:
  * The elementwise gate folds into the weight matrix: V*g @ w1 = V @ (g[:,None]*w1).
  * w2 folds into w1's columns by |w2| with a sign-grouping permutation, so the
    w2-dot becomes two plain row-sums of the relu output; those are computed
    for free by the fused relu+accumulate paths on ScalarE (ACT) and VectorE.
  * V is pre-cast to fp16 on the host; the d-major (transposed) copy needed for
    the fc1 contraction is produced by the DMA xbar transpose during the load,
    so the PE runs only the essential matmuls.
  * softmax skips max-subtraction (scores are O(0.1); masked entries get an
    additive -2^32 bias so exp underflows to exactly 0); normalization happens
    once at the end on the [1, 512] pooled accumulator.
"""

import numpy as np

B, H, D, HID = 64, 2048, 512, 512
NCORES = 8
BPC = B // NCORES          # batches per core
HT = H // 128              # 16 h-tiles per batch
DC = D // 128              # 4 contraction chunks
MASK_FILL = -2.0**32 + 1.0


def _build(hp, b2val, has_bias):
    import concourse.mybir as mybir
    from concourse import bacc
    from concourse.tile import TileContext

    F32 = mybir.dt.float32
    F16 = mybir.dt.float16
    ACTF = mybir.ActivationFunctionType
    ALU = mybir.AluOpType

    nc = bacc.Bacc(trn_type="TRN2", num_devices=NCORES)

    VB = nc.dram_tensor("VB", (BPC, H, D), F16, kind="ExternalInput")
    GT = nc.dram_tensor("GT", (BPC, 128, DC), F32, kind="ExternalInput")
    MB = nc.dram_tensor("MB", (BPC, 128, HT), F32, kind="ExternalInput")
    WA = nc.dram_tensor("WA", (D, HID), F32, kind="ExternalInput")
    if has_bias:
        BI = nc.dram_tensor("BI", (1, HID), F32, kind="ExternalInput")
    OUT = nc.dram_tensor("OUT", (BPC, D), F32, kind="ExternalOutput")

    with TileContext(nc) as tc:
        with (
            tc.tile_pool(name="const", bufs=1) as cpool,
            tc.tile_pool(name="v", bufs=2) as vpool,
            tc.tile_pool(name="vt", bufs=3 * DC) as vtpool,
            tc.tile_pool(name="w12", bufs=2) as wpool,
            tc.tile_pool(name="small", bufs=2) as spool,
            tc.tile_pool(name="scr", bufs=2) as scrpool,
            tc.tile_pool(name="fin", bufs=2) as finpool,
            tc.tile_pool(name="fc1_ps", bufs=2, space="PSUM") as fc1ps,
            tc.tile_pool(name="tot_ps", bufs=2, space="PSUM") as totps,
            tc.tile_pool(name="acc_ps", bufs=2, space="PSUM") as accps,
        ):
            # ---- one-time constants ----
            ones_col = cpool.tile([128, 1], F16, tag="ones")
            nc.vector.memset(ones_col, 1.0)

            # WA as [128, DC*HID]: chunk c at cols [c*HID, (c+1)*HID)
            wabs = cpool.tile([128, DC * HID], F32, tag="wabs")
            nc.sync.dma_start(
                out=wabs.rearrange("p (c n) -> p c n", c=DC),
                in_=WA.ap().rearrange("(c p) n -> p c n", p=128),
            )
            if has_bias:
                ones_row = cpool.tile([1, 128], F16, tag="orr")
                nc.vector.memset(ones_row, 1.0)
                bias_sb = cpool.tile([1, HID], F16, tag="bias")
                bias_f = cpool.tile([1, HID], F32, tag="biasf")
                nc.sync.dma_start(out=bias_f, in_=BI.ap())
                nc.vector.tensor_copy(bias_sb, bias_f)

            # ---- all batches' gate columns and mask biases in two DMAs ----
            gall = cpool.tile([128, BPC * DC], F32, tag="gall")
            nc.sync.dma_start(
                out=gall.rearrange("p (b c) -> p b c", b=BPC),
                in_=GT.ap().rearrange("b p c -> p b c"),
            )
            mall = cpool.tile([128, BPC * HT], F32, tag="mall")
            nc.sync.dma_start(
                out=mall.rearrange("p (b j) -> p b j", b=BPC),
                in_=MB.ap().rearrange("b p j -> p b j"),
            )
            # one staging tile for all 8 outputs; single store at the end
            oball = cpool.tile([1, BPC * D], F32, tag="oball")

            for bi in range(BPC):
                gcol = gall[:, bi * DC:(bi + 1) * DC]
                mb = mall[:, bi * HT:(bi + 1) * HT]

                # ---- gate the packed weights: W12[d, :] = g[d] * Wabs[d, :] ----
                w12 = wpool.tile([128, DC * HID], F16, tag="w12")
                for c in range(DC):
                    nc.vector.tensor_scalar_mul(
                        w12[:, c * HID:(c + 1) * HID],
                        wabs[:, c * HID:(c + 1) * HID],
                        gcol[:, c:c + 1],
                    )

                # ---- V[bi]^T via DMA xbar transpose: [128 d, 2048 tok] per chunk ----
                vts = []
                for c in range(DC):
                    vt = vtpool.tile([128, H], F16, tag="vt")
                    nc.sync.dma_start(
                        out=vt,
                        in_=VB.ap()[bi, :, c * 128:(c + 1) * 128],
                        transpose=True,
                    )
                    vts.append(vt)

                # ---- V[bi] natural [128 tok, j, d] (pass-2 rhs), 4 DMAs ----
                v_all = vpool.tile([128, HT * D], F16, tag="v")
                v3 = v_all.rearrange("p (j d) -> p j d", j=HT)
                for q in range(4):
                    nc.gpsimd.dma_start(
                        out=v3[:, 4 * q:4 * q + 4, :],
                        in_=VB.ap()[bi, 512 * q:512 * (q + 1), :]
                            .rearrange("(j p) d -> p j d", p=128),
                    )

                sp = spool.tile([128, HT], F32, tag="sp")
                sn = spool.tile([128, HT], F32, tag="sn")
                if hp == 0:
                    nc.vector.memset(sp, 0.0)
                if hp == HID:
                    nc.vector.memset(sn, 0.0)

                # ---- fc1 + fused relu/rowsum per tok-tile ----
                for j in range(HT):
                    fc1 = fc1ps.tile([128, HID], F32, tag="fc1")
                    for c in range(DC):
                        nc.tensor.matmul(
                            out=fc1,
                            lhsT=vts[c][:, j * 128:(j + 1) * 128],
                            rhs=w12[:, c * HID:(c + 1) * HID],
                            start=(c == 0),
                            stop=(c == DC - 1) and not has_bias,
                        )
                    if has_bias:
                        nc.tensor.matmul(
                            out=fc1, lhsT=ones_row, rhs=bias_sb,
                            start=False, stop=True,
                        )
                    # positive-w2 half on ACT (fused relu+rowsum)...
                    if hp > 0:
                        scra = scrpool.tile([128, HID], F16, tag="scra")
                        nc.scalar.activation(
                            out=scra[:, :hp], in_=fc1[:, :hp], func=ACTF.Relu,
                            accum_out=sp[:, j:j + 1],
                        )
                    # ...negative-w2 half on DVE (max(x,0) + add-reduce)
                    if hp < HID:
                        scrd = scrpool.tile([128, HID], F16, tag="scrd")
                        nc.vector.tensor_scalar(
                            out=scrd[:, hp:], in0=fc1[:, hp:],
                            scalar1=0.0, scalar2=None,
                            op0=ALU.max, op1=ALU.add,
                            accum_out=sn[:, j:j + 1],
                        )

                # ---- scores -> masked -> exp ----
                sc = spool.tile([128, HT], F32, tag="sc")
                nc.vector.tensor_sub(sc, sp, sn)
                scm = spool.tile([128, HT], F32, tag="scm")
                nc.vector.tensor_add(scm, sc, mb)
                alpha = spool.tile([128, HT], F16, tag="alpha")
                nc.scalar.activation(
                    out=alpha, in_=scm, func=ACTF.Exp, bias=float(b2val),
                )

                # ---- denominator: sum over all tokens ----
                tot = totps.tile([1, HT], F32, tag="tot")
                nc.tensor.matmul(out=tot, lhsT=ones_col, rhs=alpha,
                                 start=True, stop=True)
                tot_sb = finpool.tile([1, 1], F32, tag="tot_sb")
                nc.vector.tensor_reduce(
                    tot_sb, tot, axis=mybir.AxisListType.X, op=ALU.add,
                )
                inv = finpool.tile([1, 1], F32, tag="inv")
                nc.vector.reciprocal(inv, tot_sb)

                # ---- pass 2: acc[1, d] = sum_j alpha[:, j]^T @ V_j ----
                acc = accps.tile([1, D], F32, tag="acc")
                for j in range(HT):
                    nc.tensor.matmul(
                        out=acc,
                        lhsT=alpha[:, j:j + 1],
                        rhs=v3[:, j, :],
                        start=(j == 0),
                        stop=(j == HT - 1),
                    )
                nc.vector.tensor_scalar_mul(
                    oball[:, bi * D:(bi + 1) * D], acc, inv)

            nc.sync.dma_start(
                out=OUT.ap().rearrange("b d -> (b d)").rearrange("(o f) -> o f", o=1), in_=oball)

    nc.finalize()
    return nc


def _prep(K, V, mask, W, w1, b1, w2, b2):
    """Host-side input marshalling (no heavy compute)."""
    import ml_dtypes

    K = np.asarray(K, dtype=np.float32)
    V = np.asarray(V, dtype=np.float32)
    mask = np.asarray(mask)
    W = np.asarray(W, dtype=np.float32)
    w1 = np.asarray(w1, dtype=np.float32)
    b1 = np.asarray(b1, dtype=np.float32)
    w2 = np.asarray(w2, dtype=np.float32).reshape(-1)
    b2 = np.asarray(b2, dtype=np.float32).reshape(-1)

    Vb = np.ascontiguousarray(V.astype(np.float16))

    g = np.diagonal(W).astype(np.float32) * K          # [B, D]
    pos = w2 >= 0.0
    perm = np.argsort(~pos, kind="stable")             # positives first
    hp = int(pos.sum())
    wabs = (w1[:, perm] * np.abs(w2[perm])[None, :]).astype(np.float32)
    bias12 = (b1[perm] * np.abs(w2[perm])).astype(np.float32)
    has_bias = bool(np.any(bias12 != 0.0))

    # g arranged [B, 128, DC] so chunk c sits in column c (partition-major)
    gt = np.ascontiguousarray(g.reshape(B, DC, 128).transpose(0, 2, 1))
    # additive mask bias [B, 128, HT]: token j*128+p -> [p, j]
    mbias = np.where(mask, np.float32(MASK_FILL), np.float32(0.0)).astype(np.float32)
    mbias = np.ascontiguousarray(mbias.reshape(B, HT, 128).transpose(0, 2, 1))
    return Vb, gt, mbias, wabs, bias12, has_bias, hp, float(b2[0]) if b2.size else 0.0


def kernel(K, V, mask, W, w1, b1, w2, b2):
    from concourse import bass_utils

    Vb, gt, mbias, wabs, bias12, has_bias, hp, b2val = _prep(
        K, V, mask, W, w1, b1, w2, b2
    )
    nc = _build(hp, b2val, has_bias)

    in_maps = []
    for c in range(NCORES):
        sl = slice(c * BPC, (c + 1) * BPC)
        m = {
            "VB": Vb[sl],
            "GT": gt[sl],
            "MB": mbias[sl],
            "WA": wabs,
        }
        if has_bias:
            m["BI"] = bias12.reshape(1, HID)
        in_maps.append(m)

    res = bass_utils.run_bass_kernel_spmd(nc, in_maps, core_ids=list(range(NCORES)))
    out = np.concatenate([res.results[c]["OUT"] for c in range(NCORES)], axis=0)
    return out.astype(np.float32)
